# revision 19
# baseline (speedup 1.0000x reference)
"""Trainium2 Bass kernel for nn_ActivationGATLayer (GNN message passing).

Computation (per reference):
    z = h * norm
    scores = relu(sum(z[src] * z[dst], -1))            per edge
    alpha  = segment_softmax(scores, dst)               per dst node
    agg    = segment_sum(alpha * z[src], dst)           [N, D]
    xhat   = batchnorm(agg)  (training stats, biased var)
    out[:, h*D:(h+1)*D] = relu(gamma_h*xhat + beta_h) * norm
    returns (out, e)   (e passes through unchanged)

Distribution: nodes (and their dst-edge segments) are sharded by node range
across 8 cores.  Each core's edges are grouped into windows of 64 dst nodes;
a uniform per-window tile count (max over cores) keeps the program SPMD.
Host-side sharding materializes the halo exchange: the h-rows / norm values
of each edge's endpoints (and a uint8 onehot of the window-local dst) are
staged per edge slot — pure data movement; all arithmetic happens on device.
Segment softmax+sum run as onehot matmuls on the tensor engine with PSUM
accumulation per window; softmax max-subtraction is algebraically dropped
(scores are relu'd, bounded ~15, exp can't overflow).  BatchNorm statistics
use per-feature column sums (ones-matmul) and one 8-core AllReduce.

Engine placement: DVE does the big elementwise passes (edge dot products,
alpha*z weighting); ACT does exp / PSUM flushes / final relu*norm; PE does
segment sums and stats; GPSIMD does the uint8->f32 onehot cast-DMA loads,
small per-group products and the Phase-D adds.
"""

import math
import numpy as np

# Problem constants (hardcoded per contract; kernel.py must be self-contained)
N = 50000
E_EDGES = 800000
D = 64
H = 4
EPS = 1e-5
C = 8                 # cores
NPC = N // C          # nodes per core = 6250
W = 64                # dst window size
WPC = math.ceil(NPC / W)   # windows per core = 98
PAIRS = (WPC + 1) // 2     # window pairs packed on 128 partitions = 49
TILE = 128            # edge slots per tile (partition dim)
GRP = 4               # windows per load/compute group

# debug/bisect toggles
ONEH_F32 = False      # upload onehot as f32 (HWDGE) instead of u8 cast-DMA
GP_TT = True          # use gpsimd for small tensor_tensor ops
PHASE_LIMIT = 99      # 1: stop after flush; 2: after stats; 3: full

_cache = {}


def _prep(h, norm, gamma, beta, src, dst):
    """Host-side sharding: partition edges by dst core, group into windows,
    pad to a uniform tile structure, stage per-edge endpoint rows/values."""
    h = np.asarray(h, dtype=np.float32)
    norm = np.asarray(norm, dtype=np.float32).reshape(-1)
    src = np.asarray(src).astype(np.int64)
    dst = np.asarray(dst).astype(np.int64)

    core = dst // NPC
    dloc = dst - core * NPC
    win = dloc // W
    wloc = (dloc - win * W).astype(np.int64)

    # per (core, window) counts -> uniform tile counts T[w] = max over cores
    cw = core * WPC + win
    counts = np.bincount(cw, minlength=C * WPC).reshape(C, WPC)
    T = np.maximum(np.ceil(counts / TILE).astype(np.int64).max(axis=0), 1)  # [WPC]
    off = np.concatenate([[0], np.cumsum(T)])  # [WPC+1]
    T_sum = int(off[-1])

    in_maps = []
    ones = np.ones((128, 1), dtype=np.float32)
    grow = np.asarray(gamma, dtype=np.float32).reshape(1, H * D)
    brow = np.asarray(beta, dtype=np.float32).reshape(1, H * D)

    for c in range(C):
        sel = np.nonzero(core == c)[0]
        ordr = np.argsort(win[sel], kind="stable")
        es = sel[ordr]
        wse = win[es]
        cnt = counts[c]
        starts = np.concatenate([[0], np.cumsum(cnt)])[:-1]
        rank = np.arange(len(es)) - starts[wse]
        g = off[wse] * TILE + rank          # global slot id
        nslot = T_sum * TILE

        hsrc = np.zeros((nslot, D), dtype=np.float32)
        hdst = np.zeros((nslot, D), dtype=np.float32)
        nsrc = np.zeros(nslot, dtype=np.float32)
        ndst = np.zeros(nslot, dtype=np.float32)
        oneh = np.zeros((nslot, W), dtype=np.uint8)
        hsrc[g] = h[src[es]]
        hdst[g] = h[dst[es]]
        nsrc[g] = norm[src[es]]
        ndst[g] = norm[dst[es]]
        oneh[g, wloc[es]] = 1

        # reshape slot-major [T_sum*128, width] -> [128, T_sum*width]
        def to_sb(a, width):
            return np.ascontiguousarray(
                a.reshape(T_sum, TILE, width).transpose(1, 0, 2).reshape(TILE, T_sum * width)
            )

        normw = np.zeros((128, PAIRS), dtype=np.float32)
        nloc = np.arange(NPC)
        wn = nloc // W
        jn = nloc - wn * W
        normw[(wn % 2) * 64 + jn, wn // 2] = norm[c * NPC + nloc]

        in_maps.append({
            "hsrc": to_sb(hsrc, D),
            "hdst": to_sb(hdst, D),
            "nsrc": to_sb(nsrc[:, None], 1),
            "ndst": to_sb(ndst[:, None], 1),
            "oneh": to_sb(oneh.astype(np.float32) if ONEH_F32 else oneh, W),
            "normw": normw,
            "grow": grow,
            "brow": brow,
            "ones": ones,
        })
    return tuple(T.tolist()), off, in_maps


def _build_program(T, reps=1, loop_reps=None):
    """Build the SPMD Bass program for tile structure T (uniform across cores).

    loop_reps: timing-only variant — wraps the whole computation in a hardware
    For_i loop (and bypasses the AllReduce, which can't sit in control flow).
    """
    from concourse import bass, bacc, tile, mybir

    WPCn = len(T)
    off = [0]
    for x in T:
        off.append(off[-1] + x)
    T_sum = off[-1]
    DP = D + 1  # agg block width: [agg | den]

    f32 = mybir.dt.float32
    u8 = mybir.dt.uint8
    AF = mybir.ActivationFunctionType
    OP = mybir.AluOpType

    nc = bacc.Bacc("TRN2", target_bir_lowering=False, debug=False, num_devices=C)
    hsrc_d = nc.dram_tensor("hsrc", [128, T_sum * D], f32, kind="ExternalInput").ap()
    hdst_d = nc.dram_tensor("hdst", [128, T_sum * D], f32, kind="ExternalInput").ap()
    nsrc_d = nc.dram_tensor("nsrc", [128, T_sum], f32, kind="ExternalInput").ap()
    ndst_d = nc.dram_tensor("ndst", [128, T_sum], f32, kind="ExternalInput").ap()
    oneh_d = nc.dram_tensor(
        "oneh", [128, T_sum * W], f32 if ONEH_F32 else u8, kind="ExternalInput"
    ).ap()
    normw_d = nc.dram_tensor("normw", [128, PAIRS], f32, kind="ExternalInput").ap()
    grow_d = nc.dram_tensor("grow", [1, H * D], f32, kind="ExternalInput").ap()
    brow_d = nc.dram_tensor("brow", [1, H * D], f32, kind="ExternalInput").ap()
    ones_d = nc.dram_tensor("ones", [128, 1], f32, kind="ExternalInput").ap()
    out_d = nc.dram_tensor("out", [128, PAIRS * H * D], f32, kind="ExternalOutput").ap()

    groups = [list(range(s, min(s + GRP, WPCn))) for s in range(0, WPCn, GRP)]
    Tg_max = max(sum(T[w] for w in grp) for grp in groups)

    def rep(ap, n):
        # [P, M] -> [P, n, M] with stride-0 repeat
        P, M = ap.shape
        return ap.rearrange("p (a b) -> p a b", a=1).to_broadcast([P, n, M])

    with tile.TileContext(nc) as tc:
        with (
            tc.tile_pool(name="const", bufs=1) as cp,
            tc.tile_pool(name="ld", bufs=2) as ldp,
            tc.tile_pool(name="work", bufs=2) as wkp,
            tc.tile_pool(name="small", bufs=3) as smp,
            tc.tile_pool(name="flush", bufs=2) as flp,
            tc.tile_pool(name="psum", bufs=4, space="PSUM") as pp,
            tc.tile_pool(name="statps", bufs=1, space="PSUM") as sp,
            tc.tile_pool(name="dram", bufs=1, space="DRAM") as dp,
        ):
            # ---- Phase A: persistent loads ----
            nsrc = cp.tile([128, T_sum], f32)
            ndst = cp.tile([128, T_sum], f32)
            normw = cp.tile([128, PAIRS], f32)
            grow = cp.tile([1, H * D], f32)
            brow = cp.tile([1, H * D], f32)
            ones = cp.tile([128, 1], f32)
            nc.sync.dma_start(out=nsrc[:], in_=nsrc_d[:])
            nc.sync.dma_start(out=ndst[:], in_=ndst_d[:])
            nc.sync.dma_start(out=normw[:], in_=normw_d[:])
            nc.sync.dma_start(out=grow[:], in_=grow_d[:])
            nc.sync.dma_start(out=brow[:], in_=brow_d[:])
            nc.sync.dma_start(out=ones[:], in_=ones_d[:])
            nn = cp.tile([128, T_sum], f32)
            nc.vector.tensor_tensor(out=nn[:], in0=nsrc[:], in1=ndst[:], op=OP.mult)

            # agg65 stores per-window [aggU | den] blocks, window pairs
            # packed on the partition axis; aggn holds normalized agg
            agg65 = cp.tile([128, PAIRS * DP], f32)
            aggn_all = cp.tile([128, PAIRS * D], f32)
            rec_all = cp.tile([128, PAIRS], f32)
            gamma_b = cp.tile([128, H * D], f32)
            beta_b = cp.tile([128, H * D], f32)
            nc.gpsimd.partition_broadcast(gamma_b[:], grow[:])
            nc.gpsimd.partition_broadcast(beta_b[:], brow[:])

            def body(no_cc=False):
                # ---- Phase B: per-group edge processing ----
                sum_ps = sp.tile([1, D], f32, tag="sum")
                sumsq_ps = sp.tile([1, D], f32, tag="sumsq")
                for grp in groups:
                    g0 = off[grp[0]]
                    Tg = sum(T[w] for w in grp)
                    hsrc_t = ldp.tile([128, Tg_max, D], f32, tag="hsrc")
                    hdst_t = ldp.tile([128, Tg_max, D], f32, tag="hdst")
                    O = ldp.tile([128, Tg_max, W], f32, tag="O")
                    nc.sync.dma_start(
                        out=hsrc_t[:, 0:Tg, :], in_=hsrc_d[:, g0 * D:(g0 + Tg) * D]
                    )
                    nc.sync.dma_start(
                        out=hdst_t[:, 0:Tg, :], in_=hdst_d[:, g0 * D:(g0 + Tg) * D]
                    )
                    if ONEH_F32:
                        nc.sync.dma_start(
                            out=O[:, 0:Tg, :], in_=oneh_d[:, g0 * W:(g0 + Tg) * W]
                        )
                    else:
                        # uint8 -> f32 cast during DMA (SWDGE)
                        nc.gpsimd.dma_start(
                            out=O[:, 0:Tg, :], in_=oneh_d[:, g0 * W:(g0 + Tg) * W]
                        )
                    # in-place: hdst_t becomes the elementwise product
                    nc.vector.tensor_tensor(
                        out=hdst_t[:, 0:Tg, :], in0=hsrc_t[:, 0:Tg, :],
                        in1=hdst_t[:, 0:Tg, :], op=OP.mult,
                    )
                    s0 = smp.tile([128, Tg_max], f32, tag="s0")
                    nc.vector.tensor_reduce(
                        out=s0[:, 0:Tg], in_=hdst_t[:, 0:Tg, :],
                        axis=mybir.AxisListType.X, op=OP.add,
                    )
                    s1 = smp.tile([128, Tg_max], f32, tag="s1")
                    (nc.gpsimd if GP_TT else nc.vector).tensor_tensor(
                        out=s1[:, 0:Tg], in0=s0[:, 0:Tg], in1=nn[:, g0:g0 + Tg],
                        op=OP.mult,
                    )
                    sr = smp.tile([128, Tg_max], f32, tag="sr")
                    nc.scalar.activation(sr[:, 0:Tg], s1[:, 0:Tg], AF.Relu)
                    ex = smp.tile([128, Tg_max], f32, tag="ex")
                    nc.scalar.activation(ex[:, 0:Tg], sr[:, 0:Tg], AF.Exp)
                    wgt = smp.tile([128, Tg_max], f32, tag="wgt")
                    (nc.gpsimd if GP_TT else nc.vector).tensor_tensor(
                        out=wgt[:, 0:Tg], in0=ex[:, 0:Tg], in1=nsrc[:, g0:g0 + Tg],
                        op=OP.mult,
                    )
                    # vals = [wgt * hsrc | ex]
                    vals = wkp.tile([128, Tg_max, D + 1], f32, tag="vals")
                    nc.vector.tensor_tensor(
                        out=vals[:, 0:Tg, 0:D],
                        in0=hsrc_t[:, 0:Tg, :],
                        in1=rep(wgt[:, 0:Tg], D).rearrange("p a b -> p b a"),
                        op=OP.mult,
                    )
                    nc.vector.tensor_copy(out=vals[:, 0:Tg, D], in_=ex[:, 0:Tg])
                    # segment sums via onehot matmul, PSUM-accumulated per window
                    for w in grp:
                        lt = off[w] - g0
                        poff = (w % 2) * 64
                        wp = w // 2
                        ps = pp.tile([W, D + 1], f32, tag="ps")
                        for t in range(T[w]):
                            nc.tensor.matmul(
                                out=ps[:],
                                lhsT=O[:, lt + t, :],
                                rhs=vals[:, lt + t, :],
                                start=(t == 0),
                                stop=(t == T[w] - 1),
                            )
                        nc.scalar.activation(
                            agg65[poff:poff + W, wp * DP:(wp + 1) * DP],
                            ps[:], AF.Copy,
                        )
                    if PHASE_LIMIT < 1.3:
                        continue
                    # reciprocal of denominators for this group's pair columns
                    wp0, wp1 = grp[0] // 2, grp[-1] // 2 + 1
                    denv = agg65[:].rearrange("p (a b) -> p a b", b=DP)[:, wp0:wp1, D]
                    dent = smp.tile([128, 2], f32, tag="dent")
                    ncols = wp1 - wp0
                    nc.vector.tensor_scalar_add(dent[:, 0:ncols], denv, 1e-30)
                    nc.vector.reciprocal(rec_all[:, wp0:wp1], dent[:, 0:ncols])
                    # normalize agg, then stats on the (O(1)-magnitude)
                    # normalized values: per-feature column sums via ones-matmul
                    # normalize agg (per window), then batchnorm stats per
                    # pair-column over all 128 partitions
                    for w in grp:
                        poff = (w % 2) * 64
                        wp = w // 2
                        aggU = agg65[poff:poff + W, wp * DP:wp * DP + D]
                        aggn = aggn_all[poff:poff + W, wp * D:(wp + 1) * D]
                        nc.vector.tensor_scalar_mul(
                            aggn, aggU, rec_all[poff:poff + W, wp:wp + 1]
                        )
                    if PHASE_LIMIT < 2:
                        continue
                    for wp in range(wp0, wp1):
                        aggp = aggn_all[:, wp * D:(wp + 1) * D]
                        sq_t = flp.tile([128, D], f32, tag="sq_t")
                        nc.scalar.activation(sq_t[:], aggp, AF.Square)
                        nc.tensor.matmul(
                            out=sum_ps[:], lhsT=ones[:], rhs=aggp,
                            start=(wp == 0), stop=(wp == PAIRS - 1),
                        )
                        nc.tensor.matmul(
                            out=sumsq_ps[:], lhsT=ones[:], rhs=sq_t[:],
                            start=(wp == 0), stop=(wp == PAIRS - 1),
                        )

                if PHASE_LIMIT < 3:
                    nc.sync.dma_start(
                        out=out_d[:, 0:PAIRS * DP], in_=agg65[:]
                    )
                    return
                # ---- Phase C: stats AllReduce + fused affine prep ----
                st = smp.tile([1, 2 * D], f32, tag="st")
                nc.vector.tensor_copy(out=st[:, 0:D], in_=sum_ps[:])
                nc.vector.tensor_copy(out=st[:, D:2 * D], in_=sumsq_ps[:])
                cc_in = dp.tile([1, 2 * D], f32, tag="cci")
                cc_out = dp.tile([1, 2 * D], f32, tag="cco")
                nc.gpsimd.dma_start(out=cc_in[:], in_=st[:])
                if no_cc:
                    # timing-only build: collectives can't sit inside control
                    # flow; skipped AllReduce is added back by the harness
                    cc_out2 = cc_in
                else:
                    nc.gpsimd.collective_compute(
                        "AllReduce",
                        OP.add,
                        replica_groups=[list(range(C))],
                        ins=[cc_in.opt()],
                        outs=[cc_out.opt()],
                    )
                    cc_out2 = cc_out
                stg = smp.tile([1, 2 * D], f32, tag="stg")
                nc.sync.dma_start(out=stg[:], in_=cc_out2[:])
                mean = smp.tile([1, D], f32, tag="mean")
                nc.scalar.activation(mean[:], stg[:, 0:D], AF.Copy, scale=1.0 / N)
                msq = smp.tile([1, D], f32, tag="msq")
                nc.scalar.activation(msq[:], stg[:, D:2 * D], AF.Copy, scale=1.0 / N)
                var = smp.tile([1, D], f32, tag="var")
                nc.vector.tensor_tensor(out=var[:], in0=mean[:], in1=mean[:], op=OP.mult)
                nc.vector.tensor_tensor(out=var[:], in0=msq[:], in1=var[:], op=OP.subtract)
                veps = smp.tile([1, D], f32, tag="veps")
                nc.vector.tensor_scalar_add(veps[:], var[:], EPS)
                sd = smp.tile([1, D], f32, tag="sd")
                nc.scalar.activation(sd[:], veps[:], AF.Sqrt)
                inv = smp.tile([1, D], f32, tag="inv")
                nc.vector.reciprocal(inv[:], sd[:])
                minv_row = smp.tile([1, 2 * H * D], f32, tag="minv_row")
                nc.vector.tensor_copy(
                    out=minv_row[:, 0:H * D].rearrange("p (a b) -> p a b", b=D),
                    in_=rep(mean[:], H),
                )
                nc.vector.tensor_copy(
                    out=minv_row[:, H * D:].rearrange("p (a b) -> p a b", b=D),
                    in_=rep(inv[:], H),
                )
                minv_b = smp.tile([128, 2 * H * D], f32, tag="minv_b")
                nc.gpsimd.partition_broadcast(minv_b[:], minv_row[:])
                A_cat = smp.tile([128, H * D], f32, tag="A_cat")
                nc.vector.tensor_tensor(
                    out=A_cat[:], in0=gamma_b[:], in1=minv_b[:, H * D:], op=OP.mult
                )
                B_cat = smp.tile([128, H * D], f32, tag="B_cat")
                nc.vector.tensor_tensor(
                    out=B_cat[:], in0=minv_b[:, 0:H * D], in1=A_cat[:], op=OP.mult
                )
                nc.vector.tensor_tensor(
                    out=B_cat[:], in0=beta_b[:], in1=B_cat[:], op=OP.subtract
                )

                # ---- Phase D: per-node output ----
                for wp in range(PAIRS):
                    t1 = flp.tile([128, H * D], f32, tag="t1")
                    nc.vector.tensor_tensor(
                        out=t1[:].rearrange("p (a b) -> p a b", b=D),
                        in0=rep(aggn_all[:, wp * D:(wp + 1) * D], H),
                        in1=A_cat[:].rearrange("p (a b) -> p a b", b=D),
                        op=OP.mult,
                    )
                    t2 = flp.tile([128, H * D], f32, tag="t2")
                    (nc.gpsimd if GP_TT else nc.vector).tensor_tensor(
                        out=t2[:], in0=t1[:], in1=B_cat[:], op=OP.add
                    )
                    out_t = flp.tile([128, H * D], f32, tag="out_t")
                    nc.scalar.activation(
                        out_t[:], t2[:], AF.Relu, scale=normw[:, wp:wp + 1]
                    )
                    nc.sync.dma_start(
                        out=out_d[:, wp * H * D:(wp + 1) * H * D], in_=out_t[:]
                    )

            if loop_reps:
                with tc.For_i(0, loop_reps, 1) as _i:
                    body(no_cc=True)
            else:
                for r in range(reps):
                    body(no_cc=False)
    nc.compile()
    return nc


def _get_runner(T, reps=1):
    key = (T, reps)
    if key not in _cache:
        _cache[key] = _build_program(T, reps)
    return _cache[key]


def _assemble(outs):
    h_out = np.empty((N, H * D), dtype=np.float32)
    nloc = np.arange(NPC)
    wn = nloc // W
    jn = nloc - wn * W
    p = (wn % 2) * 64 + jn
    wp = wn // 2
    cols = wp[:, None] * (H * D) + np.arange(H * D)[None, :]
    for c in range(C):
        h_out[c * NPC:(c + 1) * NPC] = outs[c]["out"][p[:, None], cols]
    return h_out


def kernel(h, e, norm, gamma, beta, src, dst):
    from concourse import bass_utils
    T, off, in_maps = _prep(h, norm, gamma, beta, src, dst)
    nc = _get_runner(T)
    res = bass_utils.run_bass_kernel_spmd(nc, in_maps, list(range(C)))
    h_out = _assemble(res.results)
    return h_out, np.asarray(e)
